# revision 2
# baseline (speedup 1.0000x reference)
"""AdvancedComRoPEAttention on 8 TRN2 NeuronCores.

Sharding: core c -> (batch b = c//4, head-group g = c%4); each core computes
4 of the 16 heads for one batch element entirely on-chip:
  - q/k/v projections (bf16 matmuls, f32 PSUM accumulation over hidden)
  - RoPE + bias-row-mean add fused into DVE elementwise ops
  - scores = q@k^T + bias, with the additive bias (incl. causal -1e30 mask)
    folded into the matmul as an extra identity-weighted contraction term
  - softmax without max-subtraction (scores are ~N(0,1) here, exp is safe),
    row sums via the activation engine's accum_out
  - attention output and its Wo projection partial sum
Host side: shard/transposes/bf16 casts in, sum partials + concat heads out.

Self-contained: hardcodes all shapes for B=2, S=2048, H=2048, 16 heads.
"""

import math
import sys

for _p in ("/opt/trn_rl_repo",):
    if _p not in sys.path:
        sys.path.insert(0, _p)

import numpy as np
import ml_dtypes

import concourse.bass as bass
import concourse.mybir as mybir
from concourse import bacc
from concourse.tile import TileContext
from concourse.bass_utils import run_bass_kernel_spmd

BF16 = mybir.dt.bfloat16
F32 = mybir.dt.float32
NPBF16 = ml_dtypes.bfloat16

B = 2
S = 2048
HID = 2048
NH = 16
D = 128
NHL = 4          # heads per core
HL = NHL * D     # 512, local head width
NCORES = 8
ST = 512         # free-dim tile
NST = S // ST    # 4
NSB = S // 128   # 16 s-blocks
NEC = HID // 128  # 16 contraction chunks over hidden
ROPE_BASE = 10000.0
MASK = -1.0e30


def build_nc():
    nc = bacc.Bacc("TRN2", target_bir_lowering=False, debug=False, num_devices=NCORES)

    xT_d = nc.declare_dram_parameter("xT", [HID, S], BF16, isOutput=False)
    wqT_d = nc.declare_dram_parameter("wqT", [HID, HL], BF16, isOutput=False)
    wkT_d = nc.declare_dram_parameter("wkT", [HID, HL], BF16, isOutput=False)
    wvT_d = nc.declare_dram_parameter("wvT", [HID, HL], BF16, isOutput=False)
    woT_d = nc.declare_dram_parameter("woT", [HL, HID], BF16, isOutput=False)
    bias_d = nc.declare_dram_parameter("bias_m", [S, S], BF16, isOutput=False)
    biasT_d = nc.declare_dram_parameter("biasT_m", [S, S], BF16, isOutput=False)
    cosq_d = nc.declare_dram_parameter("cos_q", [D, S], BF16, isOutput=False)
    sinq_d = nc.declare_dram_parameter("sin_q", [D, S], BF16, isOutput=False)
    cosk_d = nc.declare_dram_parameter("cos_k", [D, S], BF16, isOutput=False)
    sink_d = nc.declare_dram_parameter("sin_k", [D, S], BF16, isOutput=False)
    bmq_d = nc.declare_dram_parameter("bm_q", [1, S], BF16, isOutput=False)
    bmk_d = nc.declare_dram_parameter("bm_k", [1, S], BF16, isOutput=False)
    id_d = nc.declare_dram_parameter("ident", [128, 128], BF16, isOutput=False)

    attn_d = nc.declare_dram_parameter("attn", [NHL, S, S], F32, isOutput=True)
    part_d = nc.declare_dram_parameter("part", [S, HID], F32, isOutput=True)

    def bcast_ap(dram_ap, parts=128):
        return bass.AP(tensor=dram_ap.tensor, offset=dram_ap.offset,
                       ap=[[0, parts]] + list(dram_ap.ap))

    with TileContext(nc) as tc:
        with tc.tile_pool(name="persist", bufs=1) as pp, \
             tc.tile_pool(name="dramp", bufs=1, space="DRAM") as dp:
            qT = pp.tile([128, NHL, S], BF16, tag="qT")
            kT = pp.tile([128, NHL, S], BF16, tag="kT")
            vA = pp.tile([128, NSB, HL], BF16, tag="vA")
            aoT = pp.tile([128, NHL, S], BF16, tag="aoT")
            ident = pp.tile([128, 128], BF16, tag="ident")
            cq = pp.tile([D, S], BF16, tag="cq")
            sq = pp.tile([D, S], BF16, tag="sq")
            ck = pp.tile([D, S], BF16, tag="ck")
            sk = pp.tile([D, S], BF16, tag="sk")
            bmq = pp.tile([128, S], BF16, tag="bmq")
            bmk = pp.tile([128, S], BF16, tag="bmk")
            scr = dp.tile([NHL, S], F32, tag="scr")  # per-head recip row sums

            nc.sync.dma_start(out=ident, in_=id_d[:, :])
            nc.sync.dma_start(out=cq, in_=cosq_d[:, :])
            nc.sync.dma_start(out=sq, in_=sinq_d[:, :])
            nc.sync.dma_start(out=ck, in_=cosk_d[:, :])
            nc.sync.dma_start(out=sk, in_=sink_d[:, :])
            nc.sync.dma_start(out=bmq, in_=bcast_ap(bmq_d[0, :]))
            nc.sync.dma_start(out=bmk, in_=bcast_ap(bmk_d[0, :]))

            # ---------------- Stage A: projections + RoPE ----------------
            with tc.tile_pool(name="wts", bufs=1) as wp, \
                 tc.tile_pool(name="xin", bufs=2) as xp, \
                 tc.tile_pool(name="ropet", bufs=3) as rp, \
                 tc.tile_pool(name="psA", bufs=4, space="PSUM") as psA:
                wq = wp.tile([128, NEC, HL], BF16, tag="wq")
                wk = wp.tile([128, NEC, HL], BF16, tag="wk")
                wv = wp.tile([128, NEC, HL], BF16, tag="wv")
                nc.sync.dma_start(out=wq, in_=wqT_d.rearrange("(c p) m -> p c m", p=128))
                nc.sync.dma_start(out=wk, in_=wkT_d.rearrange("(c p) m -> p c m", p=128))
                nc.sync.dma_start(out=wv, in_=wvT_d.rearrange("(c p) m -> p c m", p=128))

                for st in range(NST):
                    xt = xp.tile([128, NEC, ST], BF16, tag="xt")
                    nc.sync.dma_start(
                        out=xt,
                        in_=xT_d[:, st * ST:(st + 1) * ST].rearrange("(c p) m -> p c m", p=128))
                    sl = slice(st * ST, (st + 1) * ST)
                    for h in range(NHL):
                        for w_sb, outT, cosb, sinb, bmb in (
                                (wq, qT, cq, sq, bmq), (wk, kT, ck, sk, bmk)):
                            ps = psA.tile([128, ST], F32, tag="psA")
                            for c in range(NEC):
                                nc.tensor.matmul(
                                    ps, w_sb[:, c, h * D:(h + 1) * D], xt[:, c, :],
                                    start=(c == 0), stop=(c == NEC - 1))
                            csb = rp.tile([128, ST], BF16, tag="csb")
                            nc.scalar.activation(out=csb, in_=ps,
                                                 func=mybir.ActivationFunctionType.Copy)
                            swp = rp.tile([128, ST], BF16, tag="swp")
                            nc.sync.dma_start(out=swp[0:64, :], in_=csb[64:128, :])
                            nc.sync.dma_start(out=swp[64:128, :], in_=csb[0:64, :])
                            t1 = rp.tile([128, ST], BF16, tag="t1")
                            nc.vector.tensor_tensor(out=t1, in0=csb, in1=cosb[:, sl],
                                                    op=mybir.AluOpType.mult)
                            nc.vector.tensor_tensor(out=swp, in0=swp, in1=sinb[:, sl],
                                                    op=mybir.AluOpType.mult)
                            nc.vector.tensor_tensor(out=t1, in0=t1, in1=swp,
                                                    op=mybir.AluOpType.add)
                            nc.vector.tensor_tensor(out=outT[:, h, sl], in0=t1,
                                                    in1=bmb[:, sl],
                                                    op=mybir.AluOpType.add)
                    for lsb in range(4):
                        sb = st * 4 + lsb
                        ps = psA.tile([128, HL], F32, tag="psA")
                        for c in range(NEC):
                            nc.tensor.matmul(
                                ps, xt[:, c, lsb * 128:(lsb + 1) * 128], wv[:, c, :],
                                start=(c == 0), stop=(c == NEC - 1))
                        nc.scalar.activation(out=vA[:, sb, :], in_=ps,
                                             func=mybir.ActivationFunctionType.Copy)

            # ------------- Stage B: scores/softmax/attn per head -------------
            with tc.tile_pool(name="scps", bufs=4, space="PSUM") as psS, \
                 tc.tile_pool(name="avps", bufs=2, space="PSUM") as psV, \
                 tc.tile_pool(name="expp", bufs=2) as ep, \
                 tc.tile_pool(name="biasp", bufs=4) as bp, \
                 tc.tile_pool(name="ptp", bufs=3) as ptp, \
                 tc.tile_pool(name="smallp", bufs=4) as smp, \
                 tc.tile_pool(name="rtbp", bufs=2) as rtp:
                for h in range(NHL):
                    # ---- [q,k] layout: normalized attn weights to DRAM ----
                    for qi in range(NSB):
                        nkt = qi // 4 + 1
                        ec = nkt * ST
                        exp_sb = ep.tile([128, S], F32, tag="exp")
                        rsp = smp.tile([128, 4], F32, tag="rsp")
                        for kt in range(nkt):
                            ps = psS.tile([128, ST], F32, tag="scps")
                            nc.tensor.matmul(ps, qT[:, h, qi * 128:(qi + 1) * 128],
                                             kT[:, h, kt * ST:(kt + 1) * ST],
                                             start=True, stop=False)
                            bt = bp.tile([128, ST], BF16, tag="bt")
                            nc.sync.dma_start(
                                out=bt, in_=bias_d[qi * 128:(qi + 1) * 128,
                                                   kt * ST:(kt + 1) * ST])
                            nc.tensor.matmul(ps, ident, bt, start=False, stop=True)
                            nc.scalar.activation(
                                out=exp_sb[:, kt * ST:(kt + 1) * ST], in_=ps,
                                func=mybir.ActivationFunctionType.Exp,
                                accum_out=rsp[:, kt:kt + 1])
                        rs = smp.tile([128, 1], F32, tag="rs")
                        if nkt > 1:
                            nc.vector.tensor_reduce(out=rs, in_=rsp[:, 0:nkt],
                                                    axis=mybir.AxisListType.X,
                                                    op=mybir.AluOpType.add)
                            nc.vector.reciprocal(out=rs, in_=rs)
                        else:
                            nc.vector.reciprocal(out=rs, in_=rsp[:, 0:1])
                        nc.sync.dma_start(out=scr[h, qi * 128:(qi + 1) * 128],
                                          in_=rs[:, 0])
                        for kt in range(nkt):
                            slk = slice(kt * ST, (kt + 1) * ST)
                            if kt % 2 == 0:
                                nc.vector.tensor_scalar_mul(
                                    out=exp_sb[:, slk], in0=exp_sb[:, slk], scalar1=rs)
                            else:
                                nc.scalar.activation(
                                    out=exp_sb[:, slk], in_=exp_sb[:, slk],
                                    func=mybir.ActivationFunctionType.Copy, scale=rs)
                        nc.sync.dma_start(
                            out=attn_d[h, qi * 128:(qi + 1) * 128, 0:ec],
                            in_=exp_sb[:, 0:ec])
                    # ---- [k,q] layout: P^T tiles consumed by attn@v ----
                    for qt in range(NST):
                        nkb = (qt + 1) * 4
                        slq = slice(qt * ST, (qt + 1) * ST)
                        psv = psV.tile([128, ST], F32, tag="avps")
                        rtb = rtp.tile([128, ST], F32, tag="rtb")
                        nc.sync.dma_start(out=rtb, in_=bcast_ap(scr[h, slq]))
                        for kb in range(nkb):
                            ps = psS.tile([128, ST], F32, tag="scps")
                            nc.tensor.matmul(ps, kT[:, h, kb * 128:(kb + 1) * 128],
                                             qT[:, h, slq], start=True, stop=False)
                            btt = bp.tile([128, ST], BF16, tag="btt")
                            nc.sync.dma_start(
                                out=btt, in_=biasT_d[kb * 128:(kb + 1) * 128, slq])
                            nc.tensor.matmul(ps, ident, btt, start=False, stop=True)
                            pt = ptp.tile([128, ST], BF16, tag="pt")
                            nc.scalar.activation(out=pt, in_=ps,
                                                 func=mybir.ActivationFunctionType.Exp)
                            nc.tensor.matmul(psv, vA[:, kb, h * D:(h + 1) * D], pt,
                                             start=(kb == 0), stop=(kb == nkb - 1))
                        nc.vector.tensor_tensor(out=aoT[:, h, slq], in0=psv, in1=rtb,
                                                op=mybir.AluOpType.mult)

            # ---------------- Stage C: output projection ----------------
            with tc.tile_pool(name="wop", bufs=1) as wop, \
                 tc.tile_pool(name="outp", bufs=3) as op_, \
                 tc.tile_pool(name="psE", bufs=4, space="PSUM") as psE:
                wo = wop.tile([128, NHL, HID], BF16, tag="wo")
                nc.sync.dma_start(out=wo, in_=woT_d.rearrange("(c p) m -> p c m", p=128))
                for qb in range(NSB):
                    for et in range(NST):
                        ps = psE.tile([128, ST], F32, tag="psE")
                        for hc in range(NHL):
                            nc.tensor.matmul(ps, aoT[:, hc, qb * 128:(qb + 1) * 128],
                                             wo[:, hc, et * ST:(et + 1) * ST],
                                             start=(hc == 0), stop=(hc == NHL - 1))
                        ot = op_.tile([128, ST], F32, tag="ot")
                        nc.vector.tensor_copy(out=ot, in_=ps)
                        nc.sync.dma_start(
                            out=part_d[qb * 128:(qb + 1) * 128, et * ST:(et + 1) * ST],
                            in_=ot)
    nc.compile()
    return nc


_NC_CACHE = None


def _get_nc():
    global _NC_CACHE
    if _NC_CACHE is None:
        _NC_CACHE = build_nc()
    return _NC_CACHE


def make_inputs(x, Wq, Wk, Wv, Wo, bias_matrix):
    """Host-side shard prep. Returns list of 8 input dicts."""
    f = np.float32
    x = np.asarray(x, f)
    Wq, Wk, Wv, Wo = (np.asarray(w, f) for w in (Wq, Wk, Wv, Wo))
    bias_matrix = np.asarray(bias_matrix, f)

    inv = 1.0 / (ROPE_BASE ** (np.arange(0, D, 2, dtype=f) / D))
    t = np.arange(S, dtype=f)
    freqs = t[:, None] * inv[None, :]
    emb = np.concatenate([freqs, freqs], axis=1)          # [S, D]
    cosT = np.cos(emb).T.astype(f)                        # [D, S]
    sinT = np.sin(emb).T.astype(f)
    sinS = np.concatenate([-sinT[:64], sinT[64:]], axis=0)
    isd = f(1.0 / math.sqrt(D))
    bmean = (bias_matrix.mean(axis=1) * 0.1).astype(f)    # [S]

    bias_m = np.tril(bias_matrix) + np.triu(np.full((S, S), MASK, f), k=1)
    bias_mb = bias_m.astype(NPBF16)
    biasT_mb = np.ascontiguousarray(bias_m.T).astype(NPBF16)
    identb = np.eye(128, dtype=NPBF16)

    common = {
        "bias_m": bias_mb, "biasT_m": biasT_mb, "ident": identb,
        "cos_q": (cosT * isd).astype(NPBF16), "sin_q": (sinS * isd).astype(NPBF16),
        "cos_k": cosT.astype(NPBF16), "sin_k": sinS.astype(NPBF16),
        "bm_q": (bmean * isd).reshape(1, S).astype(NPBF16),
        "bm_k": bmean.reshape(1, S).astype(NPBF16),
    }
    xTb = [np.ascontiguousarray(x[b].T).astype(NPBF16) for b in range(B)]
    ins = []
    for c in range(NCORES):
        b, g = divmod(c, 4)
        sl = slice(g * HL, (g + 1) * HL)
        ins.append(dict(
            common,
            xT=xTb[b],
            wqT=np.ascontiguousarray(Wq[sl, :].T).astype(NPBF16),
            wkT=np.ascontiguousarray(Wk[sl, :].T).astype(NPBF16),
            wvT=np.ascontiguousarray(Wv[sl, :].T).astype(NPBF16),
            woT=np.ascontiguousarray(Wo[:, sl].T).astype(NPBF16),
        ))
    return ins


def assemble(results):
    attn = np.empty((B, NH, S, S), np.float32)
    out = np.zeros((B, S, HID), np.float32)
    for c in range(NCORES):
        b, g = divmod(c, 4)
        attn[b, g * NHL:(g + 1) * NHL] = results[c]["attn"]
        out[b] += results[c]["part"]
    return out, attn


def kernel(x, Wq, Wk, Wv, Wo, bias_matrix):
    nc = _get_nc()
    ins = make_inputs(x, Wq, Wk, Wv, Wo, bias_matrix)
    res = run_bass_kernel_spmd(nc, ins, core_ids=list(range(NCORES)))
    return assemble(res.results)
